# revision 1
# baseline (speedup 1.0000x reference)
"""Trainium2 Bass kernel for CRF forward-algorithm loss (logsumexp scan).

Exp-domain matmul recurrence (see kernel_v1.py docstring for the math):
    u_t = exp(emit_t - C) * (P @ u_{t-1}),  P = exp(trans), u kept [tags, batch]

v2 optimizations over v1:
  - Emissions DMA'd 8 steps per transfer, exp() applied per 8-step tile.
  - The 16 batch lanes per core are split into two groups of 8 whose matmuls
    are interleaved per weight chunk so both share one LDWEIGHTS stream (the
    PE bottleneck is streaming the 512x512 transition matrix into the array
    every step); duplicate LDWEIGHTS are deleted post-schedule.
  - Per-group PSUM banks + per-group multiplies so DVE work on one group
    hides under the other group's matmuls.
  - Renorm every 16 steps (exact bookkeeping via stored z, log on host).

Sharding: data-parallel over batch, 16 per core on 8 cores, host sums.
"""

import numpy as np
import ml_dtypes

import concourse.bass as bass
import concourse.mybir as mybir
import concourse.tile as tile
from concourse import bacc
from concourse.bass_utils import run_bass_kernel_spmd

T = 512
S = 512
B = 128
NCORES = 8
BL = B // NCORES   # 16 per core
G = 8              # batch per group (2 groups)
TC = 4
START = 510
STOP = 511
C = 7.0
R = 16
NREN = S // R      # 32
DG = 8             # steps per DMA group

F32 = mybir.dt.float32
BF16 = mybir.dt.bfloat16


def _dedup_ldweights(nc):
    removed = 0
    for blk in nc.m.functions[0].blocks:
        insts = blk.instructions
        last_w = None
        to_del = []
        for inst in insts:
            tn = type(inst).__name__
            if tn == "InstLdweights":
                sig = repr(inst.ins[0])
                si = inst.sync_info
                clean = si is None or (
                    len(si.on_wait) == 0 and len(si.on_update) == 0
                )
                if sig == last_w and clean:
                    to_del.append(inst)
                else:
                    last_w = sig
        for inst in to_del:
            insts.remove(inst)
            removed += 1
    return removed


def _build_program():
    nc = bacc.Bacc(
        "TRN2",
        target_bir_lowering=False,
        debug=False,
        enable_asserts=False,
        num_devices=NCORES,
    )

    pt_d = nc.dram_tensor("pt", [128, TC * TC * 128], BF16, kind="ExternalInput")
    pstop_d = nc.dram_tensor("pstop", [128, TC], BF16, kind="ExternalInput")
    u0_d = nc.dram_tensor("u0", [128, TC * G], BF16, kind="ExternalInput")
    em_d = nc.dram_tensor("emt", [S // DG, 128, DG * TC * 2 * G], F32,
                          kind="ExternalInput")
    fin_d = nc.dram_tensor("fin", [1, BL], F32, kind="ExternalOutput")
    zs_d = nc.dram_tensor("zs", [1, NREN * BL], F32, kind="ExternalOutput")

    with tile.TileContext(nc) as tc:
        with (
            tc.tile_pool(name="singles", bufs=1) as singles,
            tc.tile_pool(name="empool", bufs=3) as empool,
            tc.tile_pool(name="ehpool", bufs=3) as ehpool,
            tc.tile_pool(name="upool", bufs=2) as upool,
            tc.tile_pool(name="rnpool", bufs=2) as rnpool,
            tc.tile_pool(name="pspool", bufs=2, space="PSUM") as pspool,
            tc.tile_pool(name="pzpool", bufs=2, space="PSUM") as pzpool,
        ):
            ptsb = singles.tile([128, TC * TC * 128], BF16)
            nc.sync.dma_start(out=ptsb, in_=pt_d[:, :])
            pstop_sb = singles.tile([128, TC], BF16)
            nc.sync.dma_start(out=pstop_sb, in_=pstop_d[:, :])
            uA = upool.tile([128, TC * G], BF16, name="uA", tag="uA")
            nc.sync.dma_start(out=uA, in_=u0_d[:, :])
            uB = upool.tile([128, TC * G], BF16, name="uB", tag="uB")
            nc.sync.dma_start(out=uB, in_=u0_d[:, :])
            ones_sb = singles.tile([128, 1], BF16)
            nc.vector.memset(ones_sb, 1.0)
            negc_sb = singles.tile([128, 1], F32)
            nc.vector.memset(negc_sb, -C)
            zs_sb = singles.tile([1, NREN * BL], F32)

            eh8 = None
            for t in range(S):
                s = t % DG
                if s == 0:
                    gi = t // DG
                    em8 = empool.tile([128, DG * 64], F32, name="em8", tag="em")
                    nc.sync.dma_start(out=em8, in_=em_d[gi])
                    eh8 = ehpool.tile([128, DG * 64], F32, name="eh8", tag="eh")
                    nc.scalar.activation(
                        eh8, em8, mybir.ActivationFunctionType.Exp,
                        bias=negc_sb, scale=1.0,
                    )
                ehv = eh8.rearrange("p (s i g b) -> p s i g b", s=DG, i=TC, g=2)

                psA = pspool.tile([128, TC * G], F32, name="psA", tag="sa")
                psB = pspool.tile([128, TC * G], F32, name="psB", tag="sb")
                for j in range(TC):
                    for i in range(TC):
                        w = ptsb[:, (i * TC + j) * 128 : (i * TC + j + 1) * 128]
                        nc.tensor.matmul(
                            psA[:, j * G : (j + 1) * G], w,
                            uA[:, i * G : (i + 1) * G],
                            start=(i == 0), stop=(i == TC - 1),
                            skip_group_check=True,
                        )
                        nc.tensor.matmul(
                            psB[:, j * G : (j + 1) * G], w,
                            uB[:, i * G : (i + 1) * G],
                            start=(i == 0), stop=(i == TC - 1),
                            skip_group_check=True,
                        )
                uA_new = upool.tile([128, TC * G], BF16, name="uA", tag="uA")
                uB_new = upool.tile([128, TC * G], BF16, name="uB", tag="uB")
                nc.vector.tensor_mul(uA_new, psA, ehv[:, s, :, 0, :])
                nc.vector.tensor_mul(uB_new, psB, ehv[:, s, :, 1, :])

                if t % R == R - 1:
                    r = t // R
                    for g, (u_new, col0) in enumerate(((uA_new, 0), (uB_new, G))):
                        zp = pzpool.tile([1, G], F32, name="zp", tag="z")
                        for i in range(TC):
                            nc.tensor.matmul(
                                zp, ones_sb, u_new[:, i * G : (i + 1) * G],
                                start=(i == 0), stop=(i == TC - 1),
                                skip_group_check=True,
                            )
                        nc.vector.tensor_copy(
                            zs_sb[0:1, r * BL + col0 : r * BL + col0 + G], zp
                        )
                        zr = rnpool.tile([1, G], F32, name="zr", tag="zr")
                        nc.vector.reciprocal(zr, zp)
                        zb = rnpool.tile([128, G], F32, name="zb", tag="zb")
                        nc.gpsimd.partition_broadcast(zb, zr)
                        for i in range(TC):
                            nc.vector.tensor_mul(
                                u_new[:, i * G : (i + 1) * G],
                                u_new[:, i * G : (i + 1) * G], zb,
                            )
                uA, uB = uA_new, uB_new

            fin_sb = singles.tile([1, BL], F32)
            for g, (u, col0) in enumerate(((uA, 0), (uB, G))):
                finp = pzpool.tile([1, G], F32, name="finp", tag="z")
                for i in range(TC):
                    nc.tensor.matmul(
                        finp, pstop_sb[:, i : i + 1], u[:, i * G : (i + 1) * G],
                        start=(i == 0), stop=(i == TC - 1),
                        skip_group_check=True,
                    )
                nc.vector.tensor_copy(fin_sb[0:1, col0 : col0 + G], finp)
            nc.sync.dma_start(out=fin_d[0:1, :], in_=fin_sb)
            nc.sync.dma_start(out=zs_d[0:1, :], in_=zs_sb)

    n = _dedup_ldweights(nc)
    nc._ldw_removed = n
    nc.compile()
    return nc


def _prep_inputs(emissions, transitions):
    bf = ml_dtypes.bfloat16
    P = np.exp(transitions.astype(np.float32))
    PT = np.ascontiguousarray(P.T)                      # [prev, next]
    pt_host = np.ascontiguousarray(
        PT.reshape(TC, 128, TC, 128).transpose(1, 0, 2, 3)
    ).reshape(128, TC * TC * 128).astype(bf)
    pstop = np.exp(transitions[STOP].astype(np.float32))
    pstop_host = np.ascontiguousarray(pstop.reshape(TC, 128).T).astype(bf)
    u0_host = np.zeros((128, TC * G), dtype=bf)
    u0_host[START % 128, (START // 128) * G : (START // 128 + 1) * G] = 1.0

    in_maps = []
    for c in range(NCORES):
        sh = emissions[c * BL : (c + 1) * BL]           # [BL, S, T]
        # emt[gi, k, ((s, i, g, b))] = sh[g*8+b, 8*gi+s, 128*i+k]
        a = sh.transpose(1, 2, 0)                       # [t, n, bb]
        a = a.reshape(S // DG, DG, TC, 128, 2, G)       # [gi, s, i, k, g, b]
        emt = np.ascontiguousarray(a.transpose(0, 3, 1, 2, 4, 5)).reshape(
            S // DG, 128, DG * TC * 2 * G
        ).astype(np.float32)
        in_maps.append({"pt": pt_host, "pstop": pstop_host, "u0": u0_host,
                        "emt": emt})
    return in_maps


def _loss_from_outputs(results):
    total = 0.0
    for res in results:
        fin = np.asarray(res["fin"], np.float64).reshape(BL)
        zs = np.asarray(res["zs"], np.float64).reshape(NREN, BL)
        loss_b = np.log(fin) + np.log(zs).sum(axis=0) + S * C
        total += loss_b.sum()
    return np.float32(total)


def _run(inputs, **kwargs):
    emissions = np.asarray(inputs["inputs"], dtype=np.float32)
    transitions = np.asarray(inputs["transitions"], dtype=np.float32)
    assert emissions.shape == (B, S, T), emissions.shape
    nc = _build_program()
    in_maps = _prep_inputs(emissions, transitions)
    res = run_bass_kernel_spmd(nc, in_maps, core_ids=list(range(NCORES)), **kwargs)
    return _loss_from_outputs(res.results), res


def kernel(**inputs) -> np.ndarray:
    out, _ = _run(inputs)
    return out



# revision 8
# speedup vs baseline: 1.6632x; 1.6632x over previous
"""Trainium2 Bass kernel for CRF forward-algorithm loss (logsumexp scan).

Exp-domain matmul recurrence, split into two independent half-length chains
that run simultaneously and merge in the middle:

  forward :  ua_t = exp(emit_t - C) * (P  @ ua_{t-1}),  t = 0..255
  backward:  ub_{t-1} = P^T @ wb_t;  wb_{t-1} = exp(emit_{t-1} - C) * ub_{t-1},
             wb_511 = exp(emit_511 - C) * exp(trans[STOP]),  t = 511..256
  loss_b  =  log(sum_p ua_255[p] * ub_255[p]) + sum log z + 512*C

The two chains are data-independent, so each chain's PSUM-drain + semaphore +
eviction latency hides under the other chain's matmuls, and sequential depth
halves (256 rounds instead of 512 steps).

Layout/scheduling:
  - One [128, 64] f32 PSUM tile per chain per round (double-buffered banks);
    j-groups accumulate j-major (all 4 i-contributions of a group
    back-to-back) because matmul start=True pend-zeroes the whole 2KB bank —
    interleaving another group's start between a group's accumulates loses
    the partial sums.
  - Evictions are two contiguous [128, 32] DVE multiplies per chain per round
    (psum chunk * exp(emission)); the first depends only on groups j0,j1 so
    it runs under the j2/j3 matmuls and the other chain's block.
  - Renorm every 64 rounds per chain, entirely off the PE: DVE chunk-adds +
    gpsimd partition_all_reduce + DVE reciprocal + in-place multiplies.

Sharding: data-parallel over batch, 16 per core on 8 cores, host sums.
"""

import numpy as np
import ml_dtypes

import concourse.bass as bass
import concourse.bass_isa as bass_isa
import concourse.mybir as mybir
import concourse.tile as tile
from concourse import bacc
from concourse.bass_utils import run_bass_kernel_spmd

T = 512
S = 512
B = 128
NCORES = 8
BL = B // NCORES   # 16 per core
TC = 4
START = 510
STOP = 511
C = 7.0
R = 64
NRF = 4            # fwd renorms (rounds 63,127,191,255)
NRB = 3            # bwd renorms (rounds 63,127,191)
NRENT = NRF + NRB
DG = 8             # steps per DMA group
NR = S // 2        # rounds

F32 = mybir.dt.float32
BF16 = mybir.dt.bfloat16


def _build_program():
    nc = bacc.Bacc(
        "TRN2",
        target_bir_lowering=False,
        debug=False,
        enable_asserts=False,
        num_devices=NCORES,
    )

    ptf_d = nc.dram_tensor("ptf", [128, TC * TC * 128], BF16, kind="ExternalInput")
    ptb_d = nc.dram_tensor("ptb", [128, TC * TC * 128], BF16, kind="ExternalInput")
    u0_d = nc.dram_tensor("u0", [128, TC * BL], BF16, kind="ExternalInput")
    ubstop_d = nc.dram_tensor("ubstop", [128, TC * BL], BF16, kind="ExternalInput")
    em_d = nc.dram_tensor("emt", [S // DG, 128, DG * TC * BL], F32,
                          kind="ExternalInput")
    fin_d = nc.dram_tensor("fin", [1, BL], F32, kind="ExternalOutput")
    zs_d = nc.dram_tensor("zs", [1, NRENT * BL], F32, kind="ExternalOutput")

    with tile.TileContext(nc) as tc:
        with (
            tc.tile_pool(name="singles", bufs=1) as singles,
            tc.tile_pool(name="emfpool", bufs=3) as emfpool,
            tc.tile_pool(name="ehfpool", bufs=3) as ehfpool,
            tc.tile_pool(name="embpool", bufs=3) as embpool,
            tc.tile_pool(name="ehbpool", bufs=3) as ehbpool,
            tc.tile_pool(name="ufpool", bufs=2) as ufpool,
            tc.tile_pool(name="wbpool", bufs=2) as wbpool,
            tc.tile_pool(name="rnpool", bufs=2) as rnpool,
            tc.tile_pool(name="psfpool", bufs=2, space="PSUM") as psfpool,
            tc.tile_pool(name="psbpool", bufs=2, space="PSUM") as psbpool,
        ):
            ptf = singles.tile([128, TC * TC * 128], BF16)
            nc.sync.dma_start(out=ptf, in_=ptf_d[:, :])
            ptb = singles.tile([128, TC * TC * 128], BF16)
            nc.sync.dma_start(out=ptb, in_=ptb_d[:, :])
            uf = ufpool.tile([128, TC * BL], BF16, name="uf", tag="uf")
            nc.sync.dma_start(out=uf, in_=u0_d[:, :])
            ubstop = singles.tile([128, TC * BL], BF16)
            nc.sync.dma_start(out=ubstop, in_=ubstop_d[:, :])
            negc_sb = singles.tile([128, 1], F32)
            nc.vector.memset(negc_sb, -C)
            zs_sb = singles.tile([1, NRENT * BL], F32)

            def load_group(gi, empool, ehpool, nm):
                em8 = empool.tile([128, DG * TC * BL], F32, name="em" + nm,
                                  tag="em" + nm)
                nc.sync.dma_start(out=em8, in_=em_d[gi])
                eh8 = ehpool.tile([128, DG * TC * BL], F32, name="eh" + nm,
                                  tag="eh" + nm)
                nc.scalar.activation(
                    eh8, em8, mybir.ActivationFunctionType.Exp,
                    bias=negc_sb, scale=1.0,
                )
                # [p, s, i, b]
                return eh8.rearrange("p (s i b) -> p s i b", s=DG, i=TC)

            ehf = load_group(0, emfpool, ehfpool, "f")
            ehb = load_group(S // DG - 1, embpool, ehbpool, "b")

            # wb_511 = exp(trans[STOP]) * exp(emit_511 - C)
            wb = wbpool.tile([128, TC * BL], BF16, name="wb", tag="wb")
            nc.vector.tensor_mul(
                wb, ubstop,
                ehb[:, DG - 1, :, :],
            )

            def renorm(u_new, slot):
                t32 = rnpool.tile([128, 2 * BL], F32, name="t32", tag="t32")
                nc.vector.tensor_add(t32, u_new[:, 0:2 * BL],
                                     u_new[:, 2 * BL:4 * BL])
                t16 = rnpool.tile([128, BL], F32, name="t16", tag="t16")
                nc.vector.tensor_add(t16, t32[:, 0:BL], t32[:, BL:2 * BL])
                zall = rnpool.tile([128, BL], F32, name="zall", tag="zall")
                nc.gpsimd.partition_all_reduce(
                    zall, t16, 128, bass_isa.ReduceOp.add
                )
                nc.vector.tensor_copy(
                    zs_sb[0:1, slot * BL:(slot + 1) * BL], zall[0:1, :]
                )
                zr = rnpool.tile([128, BL], F32, name="zr", tag="zr")
                nc.vector.reciprocal(zr, zall)
                for i in range(TC):
                    nc.vector.tensor_mul(
                        u_new[:, i * BL:(i + 1) * BL],
                        u_new[:, i * BL:(i + 1) * BL], zr,
                    )

            psb_last = None
            for r in range(NR):
                k, s = divmod(r, DG)
                if s == 0 and 1 <= k + 1 <= 31:
                    ehf_next = load_group(k + 1, emfpool, ehfpool, "f")
                if s == 0 and 32 <= 62 - k <= 62:
                    ehb_next = load_group(62 - k, embpool, ehbpool, "b")

                tb = 510 - r                     # bwd emission step
                kb, sb = divmod(tb, DG)

                psf = psfpool.tile([128, TC * BL], F32, name="psf", tag="psf")
                psb = psbpool.tile([128, TC * BL], F32, name="psb", tag="psb")

                # fwd matmuls, j-major (a group's 4 accumulates stay together)
                for j in range(TC):
                    for i in range(TC):
                        nc.tensor.matmul(
                            psf[:, j * BL:(j + 1) * BL],
                            ptf[:, (i * TC + j) * 128:(i * TC + j + 1) * 128],
                            uf[:, i * BL:(i + 1) * BL],
                            start=(i == 0), stop=(i == TC - 1),
                            skip_group_check=True,
                        )
                # bwd matmuls
                for j in range(TC):
                    for i in range(TC):
                        nc.tensor.matmul(
                            psb[:, j * BL:(j + 1) * BL],
                            ptb[:, (i * TC + j) * 128:(i * TC + j + 1) * 128],
                            wb[:, i * BL:(i + 1) * BL],
                            start=(i == 0), stop=(i == TC - 1),
                            skip_group_check=True,
                        )

                uf_new = ufpool.tile([128, TC * BL], BF16, name="uf", tag="uf")
                for h in range(2):
                    nc.vector.tensor_mul(
                        uf_new[:, h * 2 * BL:(h + 1) * 2 * BL],
                        psf[:, h * 2 * BL:(h + 1) * 2 * BL],
                        ehf[:, s, 2 * h:2 * h + 2, :],
                    )
                if r < NR - 1:
                    wb_new = wbpool.tile([128, TC * BL], BF16, name="wb",
                                         tag="wb")
                    for h in range(2):
                        nc.vector.tensor_mul(
                            wb_new[:, h * 2 * BL:(h + 1) * 2 * BL],
                            psb[:, h * 2 * BL:(h + 1) * 2 * BL],
                            ehb[:, sb, 2 * h:2 * h + 2, :],
                        )
                else:
                    psb_last = psb

                if r % R == R - 1:
                    renorm(uf_new, r // R)
                    if r < NR - 1:
                        renorm(wb_new, NRF + r // R)

                uf = uf_new
                if r < NR - 1:
                    wb = wb_new
                if s == DG - 1:
                    if 1 <= k + 1 <= 31:
                        ehf = ehf_next
                if sb == 0 and 32 <= kb - 1:
                    ehb = ehb_next

            # merge: fin_b = sum_p ua_255[p] * ub_255[p]
            ubf = singles.tile([128, TC * BL], F32)
            nc.vector.tensor_copy(ubf, psb_last)
            prod = singles.tile([128, TC * BL], F32)
            nc.vector.tensor_mul(prod, ubf, uf)
            m32 = singles.tile([128, 2 * BL], F32)
            nc.vector.tensor_add(m32, prod[:, 0:2 * BL], prod[:, 2 * BL:4 * BL])
            m16 = singles.tile([128, BL], F32)
            nc.vector.tensor_add(m16, m32[:, 0:BL], m32[:, BL:2 * BL])
            mall = singles.tile([128, BL], F32)
            nc.gpsimd.partition_all_reduce(
                mall, m16, 128, bass_isa.ReduceOp.add
            )
            fin_sb = singles.tile([1, BL], F32)
            nc.vector.tensor_copy(fin_sb, mall[0:1, :])
            nc.sync.dma_start(out=fin_d[0:1, :], in_=fin_sb)
            nc.sync.dma_start(out=zs_d[0:1, :], in_=zs_sb)

    nc.compile()
    return nc


def _chunk128(M):
    """[512, 512] -> [128, 16*128] with chunk (i, j) at cols (i*TC+j)*128."""
    return np.ascontiguousarray(
        M.reshape(TC, 128, TC, 128).transpose(1, 0, 2, 3)
    ).reshape(128, TC * TC * 128)


def _prep_inputs(emissions, transitions):
    bf = ml_dtypes.bfloat16
    P = np.exp(transitions.astype(np.float32))          # P[n, p]
    ptf_host = _chunk128(np.ascontiguousarray(P.T)).astype(bf)  # fwd: lhsT=PT
    ptb_host = _chunk128(P).astype(bf)                           # bwd: lhsT=P
    u0_host = np.zeros((128, TC * BL), dtype=bf)
    u0_host[START % 128, (START // 128) * BL:(START // 128 + 1) * BL] = 1.0
    pstop = np.exp(transitions[STOP].astype(np.float32))  # [p]
    ubstop_host = np.ascontiguousarray(
        np.repeat(pstop.reshape(TC, 128).T[:, :, None], BL, axis=2)
    ).reshape(128, TC * BL).astype(bf)

    in_maps = []
    for c in range(NCORES):
        sh = emissions[c * BL:(c + 1) * BL]             # [BL, S, T]
        a = sh.transpose(1, 2, 0)                       # [t, n, b]
        a = a.reshape(S // DG, DG, TC, 128, BL)         # [gi, s, i, k, b]
        emt = np.ascontiguousarray(a.transpose(0, 3, 1, 2, 4)).reshape(
            S // DG, 128, DG * TC * BL
        ).astype(np.float32)
        in_maps.append({"ptf": ptf_host, "ptb": ptb_host, "u0": u0_host,
                        "ubstop": ubstop_host, "emt": emt})
    return in_maps


def _loss_from_outputs(results):
    total = 0.0
    for res in results:
        fin = np.asarray(res["fin"], np.float64).reshape(BL)
        zs = np.asarray(res["zs"], np.float64).reshape(NRENT, BL)
        loss_b = np.log(fin) + np.log(zs).sum(axis=0) + S * C
        total += loss_b.sum()
    return np.float32(total)


def _run(inputs, **kwargs):
    emissions = np.asarray(inputs["inputs"], dtype=np.float32)
    transitions = np.asarray(inputs["transitions"], dtype=np.float32)
    assert emissions.shape == (B, S, T), emissions.shape
    nc = _build_program()
    in_maps = _prep_inputs(emissions, transitions)
    res = run_bass_kernel_spmd(nc, in_maps, core_ids=list(range(NCORES)), **kwargs)
    return _loss_from_outputs(res.results), res


def kernel(**inputs) -> np.ndarray:
    out, _ = _run(inputs)
    return out
